# revision 30
# baseline (speedup 1.0000x reference)
"""Trainium2 Bass kernel for nn_Absolute_attention (sparse_attention).

Key algebraic identity: with qs[b,l,h] = sum_hd(sigmoid(xQw - exp(qb)))/HD * mask,
  attn[b,l,t,h] = qs[b,l,h] * (time[l,h,:] . time[t,h,:])
  comb[b,l,h,:] = qs[b,l,h] * time[l,h,:] @ M[b,h]   where M[b,h] = time[:,h,:]^T @ v[b,:,h,:]
so the O(L^2) attention collapses to a per-head [128,64] state matrix M.

Sharding: 8 cores, cores 0-3 <- batch 0, cores 4-7 <- batch 1; each core owns a
512-token chunk for everything except v/M, which it (redundantly) computes over
its batch's full 2048 tokens to avoid a cross-core reduction of M.

Token order is rolled per-core so that each core's chunk is always tokens
[0:512) of its rolled views -> a single SPMD program works for all cores.
"""

import functools

import numpy as np
import ml_dtypes

P = 128
B = 2
L = 2048
D = 512
H = 8
HD = 64
F = 128  # 2*TD time-feature dim per head
LIN = 1536
EPS = 1e-5
NCORES = 8
CPB = 4          # cores per batch
T = L // CPB     # 512 tokens per core chunk
KD = D // P      # 4
KL = L // P      # 16
KT = T // P      # 4
KLIN = LIN // P  # 12

BF16 = ml_dtypes.bfloat16

# If True: compute v/M over the local 512-token chunk only and AllReduce the
# tiny per-head state matrix M across the 4 cores of each batch. If False:
# each core redundantly computes v/M over its batch's full 2048 tokens.
COLLECTIVE = False


def _build_program(collective=COLLECTIVE):
    import concourse.bass as bass
    import concourse.bacc as bacc
    import concourse.mybir as mybir
    import concourse.tile as tile
    from concourse.masks import make_identity

    f32 = mybir.dt.float32
    bf16 = mybir.dt.bfloat16
    AF = mybir.ActivationFunctionType

    nc = bacc.Bacc("TRN2", target_bir_lowering=False, debug=False,
                   num_devices=NCORES)

    def din(name, shape, dt=bf16):
        return nc.dram_tensor(name, list(shape), dt, kind="ExternalInput").ap()

    KX = KT if collective else KL   # token tiles covered by the V/M phases
    LX = KX * P

    x_tok = din("x_tok", (KT, P, D), f32)          # chunk, token-major (Ob folded in)
    xT_full = din("xT_full", (KD, P, LX))          # feature-major, rolled
    time_tok = din("time_tok", (KX, P, H * F))     # rolled token-major time features
    time_featT = din("time_featT", (H, F, T))      # chunk feature-major time
    qw = din("qw", (KD, P, D))
    vw = din("vw", (KD, P, D))
    ow = din("ow", (KD, P, D))
    win = din("win", (KD, P, LIN))                 # ln1_g folded in
    wout = din("wout", (KLIN, P, D))
    e4 = din("e4", (KD, P, H))                     # block ones / HD
    e2 = din("e2", (H, D))                         # head -> 64-row expansion
    m_bias = din("m_bias", (P, H * HD), f32)       # Vb folded via sum_t time
    bias_q = din("bias_q", (KD, P, 1), f32)        # -exp(q_bias)
    h1_bias = din("h1_bias", (KLIN, P, 1), f32)    # b_in + ln1_b @ Win
    seed_f2 = din("seed_f2", (1, D))               # b_out + ln1_b
    mask_row = din("mask_row", (1, T), f32)        # attention_mask chunk
    g1_row = din("g1_row", (1, D), f32)            # ln1_g
    g2_row = din("g2_row", (1, D), f32)            # ln2_g
    b2_row = din("b2_row", (1, D), f32)            # ln2_b

    out = nc.dram_tensor("out", [KT, P, D], f32, kind="ExternalOutput").ap()
    if collective:
        cc_in = nc.dram_tensor("cc_in", [P, H * HD], f32).ap()
        cc_out = nc.dram_tensor("cc_out", [P, H * HD], f32).ap()

    def bcast_dram_row(row_ap, n=P):
        # [1, W] DRAM row -> [n, W] partition-broadcast read AP
        return bass.AP(tensor=row_ap.tensor, offset=row_ap.offset,
                       ap=[[0, n]] + list(row_ap.ap[1:]))

    with tile.TileContext(nc) as tc:
        import contextlib
        ctx = contextlib.ExitStack()
        with ctx:
            per = ctx.enter_context(tc.tile_pool(name="per", bufs=1))

            def sb(name, shape, dt=bf16):
                return per.tile(list(shape), dt, name=name, tag=name)

            # ---- resident SBUF tensors (DMA'd once) ----
            # Spread the big loads over per-engine HWDGE queues so they run
            # concurrently instead of serializing on the SP queue.
            xT_sb = sb("xT_sb", (P, KD, LX))
            vw_sb = sb("vw_sb", (P, KD, D))
            tt_sb = sb("tt_sb", (P, KX, H * F))
            qw_sb = sb("qw_sb", (P, KD, D))
            tf_sb = sb("tf_sb", (P, H, T))
            ow_sb = sb("ow_sb", (P, KD, D))
            win_sb = sb("win_sb", (P, KD, LIN))
            wout_sb = sb("wout_sb", (P, KLIN, D))
            xtok_sb = sb("xtok_sb", (P, KT, D), f32)
            e4_sb = sb("e4_sb", (P, KD, H))
            e2_sb = sb("e2_sb", (H, D))
            mb_sb = sb("mb_sb", (P, H * HD), f32)
            bq_sb = sb("bq_sb", (P, KD), f32)
            h1b_sb = sb("h1b_sb", (P, KLIN), f32)
            sf2_sb = sb("sf2_sb", (1, D))
            mask_sb = sb("mask_sb", (P, T), f32)
            g1_sb = sb("g1_sb", (P, D), f32)
            g2_sb = sb("g2_sb", (P, D), f32)
            b2_sb = sb("b2_sb", (P, D), f32)

            half = KX // 2
            # DMA issue order: per-queue FIFO staggers transfers, so each
            # queue leads with what compute needs first.
            for mc in range(LX // T):
                nc.sync.dma_start(
                    out=xT_sb[:, :, mc * T:(mc + 1) * T],
                    in_=xT_full[:, :, mc * T:(mc + 1) * T].rearrange(
                        "k p l -> p k l"))
            qtr = max(KX // 4, 1)
            nc.sync.dma_start(
                out=tt_sb[:, :qtr, :],
                in_=time_tok[:qtr].rearrange("k p w -> p k w"))
            nc.sync.dma_start(
                out=tt_sb[:, qtr:2 * qtr, :],
                in_=time_tok[qtr:2 * qtr].rearrange("k p w -> p k w"))
            nc.sync.dma_start(out=tf_sb, in_=time_featT.rearrange("h f t -> f h t"))
            nc.sync.dma_start(out=xtok_sb, in_=x_tok.rearrange("k p d -> p k d"))
            nc.sync.dma_start(out=wout_sb, in_=wout.rearrange("k p d -> p k d"))

            nc.scalar.dma_start(out=vw_sb, in_=vw.rearrange("k p d -> p k d"))
            nc.scalar.dma_start(out=qw_sb, in_=qw.rearrange("k p d -> p k d"))
            nc.scalar.dma_start(out=ow_sb, in_=ow.rearrange("k p d -> p k d"))
            nc.scalar.dma_start(
                out=tt_sb[:, 2 * qtr:3 * qtr, :],
                in_=time_tok[2 * qtr:3 * qtr].rearrange("k p w -> p k w"))
            nc.scalar.dma_start(
                out=tt_sb[:, 3 * qtr:, :],
                in_=time_tok[3 * qtr:].rearrange("k p w -> p k w"))
            nc.scalar.dma_start(out=win_sb, in_=win.rearrange("k p w -> p k w"))

            nc.gpsimd.dma_start(out=e4_sb, in_=e4.rearrange("k p h -> p k h"))
            nc.gpsimd.dma_start(out=e2_sb, in_=e2)
            nc.gpsimd.dma_start(
                out=bq_sb, in_=bias_q.rearrange("k p one -> p (k one)"))
            nc.gpsimd.dma_start(out=sf2_sb, in_=seed_f2)
            rows_sb = sb("rows_sb", (1, 4 * D), f32)  # mask|g1|g2|b2 rows
            nc.gpsimd.dma_start(out=rows_sb[:, 0:T], in_=mask_row)
            nc.gpsimd.dma_start(out=rows_sb[:, D:2 * D], in_=g1_row)
            nc.gpsimd.dma_start(out=rows_sb[:, 2 * D:3 * D], in_=g2_row)
            nc.gpsimd.dma_start(out=rows_sb[:, 3 * D:4 * D], in_=b2_row)
            nc.gpsimd.dma_start(out=mb_sb, in_=m_bias)
            nc.gpsimd.dma_start(
                out=h1b_sb, in_=h1_bias.rearrange("k p one -> p (k one)"))

            identb = sb("identb", (P, P), bf16)
            make_identity(nc, identb)
            ones_row = sb("ones_row", (1, P))
            nc.vector.memset(ones_row, 1.0)
            ones_row32 = sb("ones_row32", (1, P), f32)
            nc.vector.memset(ones_row32, 1.0)
            eps_sb = sb("eps_sb", (P, 1), f32)
            nc.vector.memset(eps_sb, EPS)
            warm_sb = sb("warm_sb", (P, 1), f32)
            nc.vector.memset(warm_sb, 1.0)

            # ---- working SBUF tensors ----
            v_sb = sb("v_sb", (P, KX, D))              # v token-major
            sig_sb = sb("sig_sb", (P, KD, T))          # sigmoid(q^T) feature-major
            qsum_sb = sb("qsum_sb", (H, T))
            qm_sb = sb("qm_sb", (P, KD, T), f32)       # expanded qsum * mask
            M_sb = sb("M_sb", (P, H * HD))             # [f, (h,hd)] state matrix
            combT_sb = sb("combT_sb", (P, KD, T))      # scaled comb^T
            z1_sb = sb("z1_sb", (P, KT, D), f32)
            xn1_sb = sb("xn1_sb", (P, KT, D), f32)
            xn1b_sb = sb("xn1b_sb", (P, KT, D))
            xn1T_sb = sb("xn1T_sb", (P, KD, T))
            gel_sb = sb("gel_sb", (P, KLIN, T))
            z2_sb = sb("z2_sb", (P, KT, D), f32)
            mv1_sb = sb("mv1_sb", (P, KT, 2), f32)
            nmr1_sb = sb("nmr1_sb", (P, KT), f32)
            nmr2_sb = sb("nmr2_sb", (P, KT), f32)
            mv2_sb = sb("mv2_sb", (P, KT, 2), f32)
            st_sb = sb("st_sb", (P, KT, 6), f32)
            st2_sb = sb("st2_sb", (P, KT, 6), f32)

            ps = ctx.enter_context(tc.tile_pool(name="ps", bufs=4, space="PSUM"))

            # build [128, W] broadcasts of the f32 parameter rows on the PE
            for dst, lo in ((mask_sb, 0), (g1_sb, D), (g2_sb, 2 * D),
                            (b2_sb, 3 * D)):
                wdt = dst.shape[-1]
                bp = ps.tile([P, D], f32, name="bp", tag="ps")
                nc.tensor.matmul(bp[:, :wdt], lhsT=ones_row32,
                                 rhs=rows_sb[:, lo:lo + wdt],
                                 start=True, stop=True)
                nc.vector.tensor_copy(dst, bp[:, :wdt])

            # ---- Phase V: v = x @ Vw (token-major) ----
            for m in range(KX):
                vp = ps.tile([P, D], f32, name="vp", tag="ps")
                for k in range(KD):
                    nc.tensor.matmul(vp, lhsT=xT_sb[:, k, m * P:(m + 1) * P],
                                     rhs=vw_sb[:, k, :],
                                     start=(k == 0), stop=(k == KD - 1))
                nc.vector.tensor_copy(v_sb[:, m, :], vp)

            # ---- Phase Q: q^T = Qw^T x^T ; sigmoid ; qsum ; expand ----
            for m in range(KD):
                qp = ps.tile([P, T], f32, name="qp", tag="ps")
                for k in range(KD):
                    nc.tensor.matmul(qp, lhsT=qw_sb[:, k, m * P:(m + 1) * P],
                                     rhs=xT_sb[:, k, 0:T],
                                     start=(k == 0), stop=(k == KD - 1))
                nc.scalar.activation(sig_sb[:, m, :], qp, AF.Sigmoid,
                                     bias=bq_sb[:, m:m + 1])
            nc.scalar.activation(warm_sb, warm_sb, AF.Sqrt, bias=eps_sb)
            qsp = ps.tile([H, T], f32, name="qsp", tag="ps")
            for k in range(KD):
                nc.tensor.matmul(qsp, lhsT=e4_sb[:, k, :], rhs=sig_sb[:, k, :],
                                 start=(k == 0), stop=(k == KD - 1))
            nc.vector.tensor_copy(qsum_sb, qsp)
            for m in range(KD):
                qep = ps.tile([P, T], f32, name="qep", tag="ps")
                nc.tensor.matmul(qep, lhsT=e2_sb[:, m * P:(m + 1) * P],
                                 rhs=qsum_sb, start=True, stop=True)
                nc.vector.tensor_mul(qm_sb[:, m, :], qep, mask_sb)

            # ---- Phase M: M[h] = time_h^T @ v_h  (+ m_bias) ----
            if collective:
                mpart_sb = sb("mpart_sb", (P, H * HD), f32)
                mred_sb = sb("mred_sb", (P, H * HD), f32)
            for h in range(H):
                mp = ps.tile([P, HD], f32, name="mp", tag="ps")
                for k in range(KX):
                    nc.tensor.matmul(mp,
                                     lhsT=tt_sb[:, k, h * F:(h + 1) * F],
                                     rhs=v_sb[:, k, h * HD:(h + 1) * HD],
                                     start=(k == 0), stop=(k == KX - 1))
                if collective:
                    nc.vector.tensor_copy(mpart_sb[:, h * HD:(h + 1) * HD], mp)
                else:
                    nc.vector.tensor_add(M_sb[:, h * HD:(h + 1) * HD], mp,
                                         mb_sb[:, h * HD:(h + 1) * HD])
            if collective:
                nc.sync.dma_start(out=cc_in, in_=mpart_sb)
                nc.gpsimd.collective_compute(
                    "AllReduce", mybir.AluOpType.add,
                    replica_groups=[[0, 1, 2, 3], [4, 5, 6, 7]],
                    ins=[cc_in], outs=[cc_out])
                nc.sync.dma_start(out=mred_sb, in_=cc_out)
                nc.vector.tensor_add(M_sb, mred_sb, mb_sb)

            # ---- Phase C: comb^T = M^T @ time^T, scaled by qsum*mask ----
            for hp in range(KD):
                cp = ps.tile([P, T], f32, name="cp", tag="ps")
                for j in range(2):
                    h = 2 * hp + j
                    nc.tensor.matmul(cp[j * HD:(j + 1) * HD, :],
                                     lhsT=M_sb[:, h * HD:(h + 1) * HD],
                                     rhs=tf_sb[:, h, :], start=True, stop=True)
                nc.vector.tensor_mul(combT_sb[:, hp, :], cp, qm_sb[:, hp, :])

            # ---- Phase O + LN1, pipelined per token tile: tile m's
            # transposes run on the PE while tile m+1's LN chain runs on
            # DVE/ACT, so the PE never drains for the whole layernorm.
            for m in range(KT):
                op = ps.tile([P, D], f32, name="op", tag="ps")
                for k in range(KD):
                    nc.tensor.matmul(op, lhsT=combT_sb[:, k, m * P:(m + 1) * P],
                                     rhs=ow_sb[:, k, :],
                                     start=(k == 0), stop=(k == KD - 1))
                nc.vector.tensor_add(z1_sb[:, m, :], op, xtok_sb[:, m, :])
                nc.vector.bn_stats(st_sb[:, m, :], z1_sb[:, m, :])
                nc.vector.bn_aggr(mv1_sb[:, m, :], st_sb[:, m, :])
            for m in range(KT):
                nc.scalar.activation(mv1_sb[:, m, 1:2], mv1_sb[:, m, 1:2],
                                     AF.Sqrt, bias=eps_sb)
                nc.vector.reciprocal(mv1_sb[:, m, 1:2], mv1_sb[:, m, 1:2])
                nc.vector.tensor_mul(nmr1_sb[:, m:m + 1], mv1_sb[:, m, 0:1],
                                     mv1_sb[:, m, 1:2])
                nc.vector.tensor_scalar_mul(nmr1_sb[:, m:m + 1],
                                            nmr1_sb[:, m:m + 1], -1.0)
                nc.scalar.activation(xn1b_sb[:, m, :], z1_sb[:, m, :],
                                     AF.Identity, bias=nmr1_sb[:, m:m + 1],
                                     scale=mv1_sb[:, m, 1:2])
                nc.scalar.activation(xn1_sb[:, m, :], z1_sb[:, m, :],
                                     AF.Identity, bias=nmr1_sb[:, m:m + 1],
                                     scale=mv1_sb[:, m, 1:2])
                for j in range(KD):
                    tp = ps.tile([P, P], bf16, name="tp", tag="ps")
                    nc.tensor.transpose(tp, xn1b_sb[:, m, j * P:(j + 1) * P],
                                        identb)
                    nc.vector.tensor_copy(xn1T_sb[:, j, m * P:(m + 1) * P], tp)
                nc.vector.tensor_mul(z1_sb[:, m, :], xn1_sb[:, m, :], g1_sb)

            # ---- FFN1 and FFN2 interleaved (k-outer on FFN2): the four
            # FFN2 accumulators fill as each gelu tile appears, so only 4
            # matmuls + the LN2 chains remain after the last gelu.
            fps = [ps.tile([P, D], f32, name=f"fp{m}", tag=f"fp{m}", bufs=1)
                   for m in range(KT)]
            for m in range(KT):
                nc.tensor.matmul(fps[m], lhsT=ones_row, rhs=sf2_sb,
                                 start=True, stop=False)
            for k in range(KLIN):
                hp1 = ps.tile([P, T], f32, name="hp1", tag="ps")
                for kk in range(KD):
                    nc.tensor.matmul(hp1, lhsT=win_sb[:, kk, k * P:(k + 1) * P],
                                     rhs=xn1T_sb[:, kk, :],
                                     start=(kk == 0), stop=(kk == KD - 1))
                nc.scalar.activation(gel_sb[:, k, :], hp1, AF.Gelu_apprx_tanh,
                                     bias=h1b_sb[:, k:k + 1])
                for m in range(KT):
                    nc.tensor.matmul(fps[m], lhsT=gel_sb[:, k, m * P:(m + 1) * P],
                                     rhs=wout_sb[:, k, :],
                                     start=False, stop=(k == KLIN - 1))

            nc.scalar.activation(warm_sb, warm_sb, AF.Sqrt, bias=eps_sb)

            # ---- residual + LN2 + store (batched stats) ----
            for m in range(KT):
                nc.vector.tensor_add(z2_sb[:, m, :], z1_sb[:, m, :], fps[m])
                nc.vector.bn_stats(st2_sb[:, m, :], z2_sb[:, m, :])
                nc.vector.bn_aggr(mv2_sb[:, m, :], st2_sb[:, m, :])
            nc.scalar.activation(mv2_sb[:, :, 1:2], mv2_sb[:, :, 1:2],
                                 AF.Sqrt, bias=eps_sb)
            nc.vector.reciprocal(mv2_sb[:, :, 1:2], mv2_sb[:, :, 1:2])
            nc.vector.tensor_mul(nmr2_sb, mv2_sb[:, :, 0], mv2_sb[:, :, 1])
            nc.vector.tensor_scalar_mul(nmr2_sb, nmr2_sb, -1.0)
            for m in range(KT):
                nc.scalar.activation(z1_sb[:, m, :], z2_sb[:, m, :],
                                     AF.Identity, bias=nmr2_sb[:, m:m + 1],
                                     scale=mv2_sb[:, m, 1:2])
                nc.vector.tensor_mul(z2_sb[:, m, :], z1_sb[:, m, :], g2_sb)
                nc.vector.tensor_add(z2_sb[:, m, :], z2_sb[:, m, :], b2_sb)
                nc.sync.dma_start(out=out[m], in_=z2_sb[:, m, :])

    nc.compile()
    return nc


@functools.lru_cache(maxsize=1)
def _get_program():
    return _build_program()


def _host_prep(inputs, collective=COLLECTIVE):
    """Build the 8 per-core input maps (numpy)."""
    LX = T if collective else L
    KX = LX // P
    x = np.ascontiguousarray(inputs["tensor"], dtype=np.float32)       # [B,L,D]
    mask = np.asarray(inputs["attention_mask"], dtype=np.float32)      # [B,L]
    angle = np.asarray(inputs["time_angle"], dtype=np.float32)         # [H,TD]
    delta = np.asarray(inputs["head_time_delta"], dtype=np.float32)    # [H]
    Qw = np.asarray(inputs["Qw"], np.float32)
    q_bias = np.asarray(inputs["q_bias"], np.float32)
    Vw = np.asarray(inputs["Vw"], np.float32)
    Vb = np.asarray(inputs["Vb"], np.float32)
    Ow = np.asarray(inputs["Ow"], np.float32)
    Ob = np.asarray(inputs["Ob"], np.float32)
    ln1_g = np.asarray(inputs["ln1_g"], np.float32)
    ln1_b = np.asarray(inputs["ln1_b"], np.float32)
    Win = np.asarray(inputs["Win"], np.float32)
    b_in = np.asarray(inputs["b_in"], np.float32)
    Wout = np.asarray(inputs["Wout"], np.float32)
    b_out = np.asarray(inputs["b_out"], np.float32)
    ln2_g = np.asarray(inputs["ln2_g"], np.float32)
    ln2_b = np.asarray(inputs["ln2_b"], np.float32)

    inv_sqrt = np.float32(1.0 / np.sqrt(HD))
    t = np.arange(L, dtype=np.float32)
    ang = (t[:, None, None] + delta[None, :, None]) * angle[None]      # [L,H,TD]
    c, s = np.cos(ang), np.sin(ang)
    time = np.concatenate([c + s, c - s], axis=-1) * inv_sqrt          # [L,H,F]

    # Shared (data-independent) buffers
    stime = time.sum(axis=0)                                           # [H,F]
    m_bias = np.einsum("hf,hd->fhd", stime, Vb.reshape(H, HD))
    m_bias = np.ascontiguousarray(m_bias.reshape(P, H * HD), np.float32)
    e4 = np.zeros((D, H), np.float32)
    for h in range(H):
        e4[h * HD:(h + 1) * HD, h] = 1.0 / HD
    e4 = e4.reshape(KD, P, H).astype(BF16)
    # e2 maps head h to the 64 (h,hd) columns: col j belongs to head j//64
    e2 = np.zeros((H, D), np.float32)
    cols = np.arange(D) // HD
    e2[cols, np.arange(D)] = 1.0
    e2 = e2.astype(BF16)

    shared = {
        "qw": np.ascontiguousarray(Qw.reshape(KD, P, D)).astype(BF16),
        "vw": np.ascontiguousarray(Vw.reshape(KD, P, D)).astype(BF16),
        "ow": np.ascontiguousarray(Ow.reshape(KD, P, D)).astype(BF16),
        "win": np.ascontiguousarray((ln1_g[:, None] * Win).reshape(KD, P, LIN)).astype(BF16),
        "wout": np.ascontiguousarray(Wout.reshape(KLIN, P, D)).astype(BF16),
        "e4": e4,
        "e2": e2,
        "m_bias": m_bias,
        "bias_q": (-np.exp(q_bias)).reshape(KD, P, 1).astype(np.float32),
        "h1_bias": (b_in + ln1_b @ Win).reshape(KLIN, P, 1).astype(np.float32),
        "seed_f2": (b_out + ln1_b).reshape(1, D).astype(BF16),
        "g1_row": ln1_g.reshape(1, D).astype(np.float32),
        "g2_row": ln2_g.reshape(1, D).astype(np.float32),
        "b2_row": ln2_b.reshape(1, D).astype(np.float32),
    }

    in_maps = []
    for c_id in range(NCORES):
        b, r = divmod(c_id, CPB)
        idx = np.concatenate([np.arange(r * T, L), np.arange(0, r * T)])
        xb = x[b][idx]                                   # [L, D] rolled
        time_r = time[idx]                               # [L, H, F] rolled
        m = {
            "x_tok": np.ascontiguousarray(
                (xb[:T] + Ob).reshape(KT, P, D), np.float32),
            "xT_full": np.ascontiguousarray(
                xb[:LX].T.reshape(KD, P, LX)).astype(BF16),
            "time_tok": np.ascontiguousarray(
                time_r[:LX].reshape(LX, H * F).reshape(KX, P, H * F)).astype(BF16),
            "time_featT": np.ascontiguousarray(
                time_r[:T].transpose(1, 2, 0)).astype(BF16),
            "mask_row": np.ascontiguousarray(
                mask[b][idx[:T]].reshape(1, T), np.float32),
        }
        m.update(shared)
        in_maps.append(m)
    return in_maps


def kernel(**inputs) -> np.ndarray:
    from concourse.bass_utils import run_bass_kernel_spmd

    nc = _get_program()
    in_maps = _host_prep(inputs)
    import concourse.mybir as mybir
    for alloc in nc.m.functions[0].allocations:
        if isinstance(alloc, mybir.MemoryLocationSet) and alloc.kind == "ExternalInput":
            nm = alloc.memorylocations[0].name
            if nm not in in_maps[0]:   # partition_id etc. supplied by runtime
                continue
            want = tuple(alloc.tensor_shape)
            got_shape = tuple(in_maps[0][nm].shape)
            assert got_shape == want, f"{nm}: {got_shape} != {want}"
    res = run_bass_kernel_spmd(nc, in_maps, core_ids=list(range(NCORES)))
    y = np.empty((B, L, D), np.float32)
    for c_id in range(NCORES):
        b, r = divmod(c_id, CPB)
        y[b, r * T:(r + 1) * T] = res.results[c_id]["out"].reshape(T, D)
    return y


if __name__ == "__main__":
    import reference

    inputs = {k: np.asarray(v) for k, v in reference.setup_inputs().items()}
    got = kernel(**inputs)
    print("kernel output", got.shape, got.dtype)
